# revision 1
# baseline (speedup 1.0000x reference)
"""MoCo grouped-queue logits kernel for Trainium2 (8 NeuronCores, Bass/Tile).

Computation (reference):
    q = l2norm(im_q @ W_q)          # [N, C]
    k = l2norm(im_k @ W_k)          # [N, C]
    l_pos[n] = q[n] . k[n]
    route[n] = (label[n] - 1) % 4
    l_neg[n, :] = q[n] @ queues[route[n]]    # [N, K]
    logits = concat([l_pos, l_neg], 1) / T   # [N, 1+K]
    labels = zeros(N)

Sharding: queues are sharded along K across the 8 cores (each core gets
[4, 128, K/8]); every core computes the projections for all N samples
(replicated; inputs are small) and its own [N, K/8] slice of l_neg.
Routing is handled by accumulating, per output tile, the four per-group
matmuls with the group's mask folded into the stationary operand:
    l_neg[n, k] = sum_g (q[n]*mask_g[n]/T) . queues[g][:, k]
which the PE accumulates natively in PSUM (start/stop flags).

Host side only reshapes/slices inputs into DMA-friendly layouts and
reassembles the output; all FLOPs run on device.
"""

import numpy as np

# Problem constants (hardcoded per contract; kernel.py must be self-contained).
N = 512          # batch
D = 2048         # input feature dim
C = 128          # embedding dim
K = 65536        # queue length
G = 4            # number of queues
T = 0.07         # softmax temperature
NCORES = 8
KSH = K // NCORES            # 8192 queue columns per core
DT = D // 128                # 16 contraction tiles for the projections
NT = N // 128                # 4 sample tiles
CW = 2048                    # queue-chunk width (columns per DMA chunk)
NKC = KSH // CW              # 4 chunks per core
NSUB = CW // 512             # 4 matmuls (N=512) per chunk

_cached = None


def _build():
    """Build + compile the Bass program once per process."""
    import concourse.tile as tile
    from concourse import bacc, mybir

    f32 = mybir.dt.float32
    AX = mybir.AxisListType
    AF = mybir.ActivationFunctionType

    nc = bacc.Bacc("TRN2", target_bir_lowering=False, debug=False,
                   num_devices=NCORES)

    # Inputs, pre-tiled on host so every DMA is partition-contiguous:
    #   imqt/imkt [128, DT*512]: [p, t*512+n] = im_x[n, t*128+p]
    #   wqt/wkt   [128, DT*128]: [p, t*128+c] = W_x[t*128+p, c]
    #   maskb     [128, G*512]:  [p, g*512+n] = 1/T if route[n]==g else 0
    #   qsh       [G, 128, KSH]: this core's K-slice of the queues
    imqt = nc.dram_tensor("imqt", [128, DT * 512], f32, kind="ExternalInput")
    imkt = nc.dram_tensor("imkt", [128, DT * 512], f32, kind="ExternalInput")
    wqt = nc.dram_tensor("wqt", [128, DT * 128], f32, kind="ExternalInput")
    wkt = nc.dram_tensor("wkt", [128, DT * 128], f32, kind="ExternalInput")
    maskb = nc.dram_tensor("maskb", [128, G * 512], f32, kind="ExternalInput")
    qsh = nc.dram_tensor("qsh", [G, 128, KSH], f32, kind="ExternalInput")
    # Outputs: lneg[nt, p, c] = l_neg[nt*128+p, c] (this core's K-slice),
    # lpos[0, n] = l_pos[n] (already scaled by 1/T).
    lneg = nc.dram_tensor("lneg", [NT, 128, KSH], f32, kind="ExternalOutput")
    lpos = nc.dram_tensor("lpos", [1, N], f32, kind="ExternalOutput")

    with tile.TileContext(nc) as tc:
        with tc.tile_pool(name="pers", bufs=1) as pers:
            # Constants for cross-partition reductions / broadcasts.
            ones_col = pers.tile([128, 1], f32)
            nc.vector.memset(ones_col[:], 1.0)
            ones_row = pers.tile([1, 128], f32)
            nc.vector.memset(ones_row[:], 1.0)

            maskb_sb = pers.tile([128, G * 512], f32)
            nc.sync.dma_start(maskb_sb[:], maskb[:])
            # Masked/scaled stationary operand for the queue matmuls:
            # qtg[:, g*512+n] = qT[:, n] * mask_g[n] * invnorm_q[n] / T
            qtg = pers.tile([128, G * 512], f32)

            # ---- Phase A: projections + norms + l_pos ----
            with tc.tile_pool(name="pa", bufs=1) as pa, \
                 tc.tile_pool(name="paps", bufs=1, space="PSUM") as paps:
                imq_sb = pa.tile([128, DT * 512], f32)
                nc.sync.dma_start(imq_sb[:], imqt[:])
                imk_sb = pa.tile([128, DT * 512], f32)
                nc.sync.dma_start(imk_sb[:], imkt[:])
                wq_sb = pa.tile([128, DT * 128], f32)
                nc.sync.dma_start(wq_sb[:], wqt[:])
                wk_sb = pa.tile([128, DT * 128], f32)
                nc.sync.dma_start(wk_sb[:], wkt[:])

                # qT[c, n] = sum_d W_q[d, c] * im_q[n, d], accumulated over
                # the 16 contraction tiles. Same for kT.
                qt_ps = paps.tile([128, 512], f32)
                kt_ps = paps.tile([128, 512], f32)
                for t in range(DT):
                    nc.tensor.matmul(qt_ps[:],
                                     wq_sb[:, t * 128:(t + 1) * 128],
                                     imq_sb[:, t * 512:(t + 1) * 512],
                                     start=(t == 0), stop=(t == DT - 1))
                for t in range(DT):
                    nc.tensor.matmul(kt_ps[:],
                                     wk_sb[:, t * 128:(t + 1) * 128],
                                     imk_sb[:, t * 512:(t + 1) * 512],
                                     start=(t == 0), stop=(t == DT - 1))

                qt_sb = pa.tile([128, 512], f32)
                nc.vector.tensor_copy(qt_sb[:], qt_ps[:])
                kt_sb = pa.tile([128, 512], f32)
                nc.vector.tensor_copy(kt_sb[:], kt_ps[:])

                # Column sums (over the partition dim) via ones-vector matmul.
                sqq = pa.tile([128, 512], f32)
                nc.vector.tensor_mul(sqq[:], qt_sb[:], qt_sb[:])
                sqk = pa.tile([128, 512], f32)
                nc.vector.tensor_mul(sqk[:], kt_sb[:], kt_sb[:])
                dqk = pa.tile([128, 512], f32)
                nc.vector.tensor_mul(dqk[:], qt_sb[:], kt_sb[:])

                ssqq_ps = paps.tile([1, 512], f32)
                nc.tensor.matmul(ssqq_ps[:], ones_col[:], sqq[:],
                                 start=True, stop=True)
                ssqk_ps = paps.tile([1, 512], f32)
                nc.tensor.matmul(ssqk_ps[:], ones_col[:], sqk[:],
                                 start=True, stop=True)
                dot_ps = paps.tile([1, 512], f32)
                nc.tensor.matmul(dot_ps[:], ones_col[:], dqk[:],
                                 start=True, stop=True)

                # inv = 1 / max(sqrt(ssq), 1e-12)
                normq = pa.tile([1, 512], f32)
                nc.scalar.activation(normq[:], ssqq_ps[:], AF.Sqrt)
                normqc = pa.tile([1, 512], f32)
                nc.vector.tensor_scalar_max(normqc[:], normq[:], 1e-12)
                invq = pa.tile([1, 512], f32)
                nc.vector.reciprocal(invq[:], normqc[:])

                normk = pa.tile([1, 512], f32)
                nc.scalar.activation(normk[:], ssqk_ps[:], AF.Sqrt)
                normkc = pa.tile([1, 512], f32)
                nc.vector.tensor_scalar_max(normkc[:], normk[:], 1e-12)
                invk = pa.tile([1, 512], f32)
                nc.vector.reciprocal(invk[:], normkc[:])

                # l_pos = dot * invq * invk / T  -> [1, 512]
                lp1 = pa.tile([1, 512], f32)
                nc.vector.tensor_mul(lp1[:], invq[:], invk[:])
                lp2 = pa.tile([1, 512], f32)
                nc.vector.tensor_mul(lp2[:], lp1[:], dot_ps[:])
                lp3 = pa.tile([1, 512], f32)
                nc.vector.tensor_scalar_mul(lp3[:], lp2[:], 1.0 / T)
                nc.sync.dma_start(lpos[:], lp3[:])

                # Broadcast invq across partitions: outer(ones_128, invq).
                invqb_ps = paps.tile([128, 512], f32)
                nc.tensor.matmul(invqb_ps[:], ones_row[:], invq[:],
                                 start=True, stop=True)

                for g in range(G):
                    m1 = pa.tile([128, 512], f32, tag="m1", bufs=2,
                                 name=f"m1_{g}")
                    nc.vector.tensor_mul(
                        m1[:], maskb_sb[:, g * 512:(g + 1) * 512], invqb_ps[:])
                    nc.vector.tensor_mul(
                        qtg[:, g * 512:(g + 1) * 512], qt_sb[:], m1[:])

            # ---- Phase C: l_neg = masked-qT^T @ queue shard ----
            with tc.tile_pool(name="qp", bufs=2) as qp, \
                 tc.tile_pool(name="sp", bufs=2) as sp, \
                 tc.tile_pool(name="cps", bufs=4, space="PSUM") as cps:
                for kc in range(NKC):
                    qch = []
                    for g in range(G):
                        qt_ = qp.tile([128, CW], f32, tag=f"qch{g}",
                                      name=f"qch{g}_{kc}")
                        nc.sync.dma_start(
                            qt_[:], qsh[g, :, kc * CW:(kc + 1) * CW])
                        qch.append(qt_)
                    for nt in range(NT):
                        stg = sp.tile([128, CW], f32, tag="stg",
                                      name=f"stg_{kc}_{nt}", bufs=3)
                        for sub in range(NSUB):
                            ps = cps.tile([128, 512], f32, tag="ps",
                                          name=f"ps_{kc}_{nt}_{sub}")
                            for g in range(G):
                                s0 = g * 512 + nt * 128
                                nc.tensor.matmul(
                                    ps[:],
                                    qtg[:, s0:s0 + 128],
                                    qch[g][:, sub * 512:(sub + 1) * 512],
                                    start=(g == 0), stop=(g == G - 1))
                            nc.vector.tensor_copy(
                                stg[:, sub * 512:(sub + 1) * 512], ps[:])
                        nc.sync.dma_start(
                            lneg[nt, :, kc * CW:(kc + 1) * CW], stg[:])

    nc.compile()
    return nc


def _get_program():
    global _cached
    if _cached is None:
        _cached = _build()
    return _cached


def _stage_inputs(im_q, im_k, W_q, W_k, queues, label):
    """Host-side reshapes into the layouts declared in _build()."""
    f32 = np.float32

    def tile_T(x):  # [N, D] -> [128, DT*512] with [p, t*512+n] = x[n, t*128+p]
        return np.ascontiguousarray(
            x.T.reshape(DT, 128, N).transpose(1, 0, 2).reshape(128, DT * N)
        ).astype(f32, copy=False)

    def tile_W(w):  # [D, C] -> [128, DT*128] with [p, t*128+c] = w[t*128+p, c]
        return np.ascontiguousarray(
            w.reshape(DT, 128, C).transpose(1, 0, 2).reshape(128, DT * C)
        ).astype(f32, copy=False)

    imqt = tile_T(np.asarray(im_q, dtype=f32))
    imkt = tile_T(np.asarray(im_k, dtype=f32))
    wqt = tile_W(np.asarray(W_q, dtype=f32))
    wkt = tile_W(np.asarray(W_k, dtype=f32))

    route = ((np.asarray(label).astype(np.int64) - 1) % G).astype(np.int64)
    mask = np.zeros((G, N), dtype=f32)
    mask[route, np.arange(N)] = 1.0 / T
    maskb = np.ascontiguousarray(
        np.broadcast_to(mask.reshape(1, G * N), (128, G * N)))

    queues = np.asarray(queues, dtype=f32)
    in_maps = []
    for i in range(NCORES):
        qshard = np.ascontiguousarray(queues[:, :, i * KSH:(i + 1) * KSH])
        in_maps.append({
            "imqt": imqt, "imkt": imkt, "wqt": wqt, "wkt": wkt,
            "maskb": maskb, "qsh": qshard,
        })
    return in_maps


def kernel(im_q, im_k, W_q, W_k, queues, label):
    from concourse.bass_utils import run_bass_kernel_spmd

    nc = _get_program()
    in_maps = _stage_inputs(im_q, im_k, W_q, W_k, queues, label)
    res = run_bass_kernel_spmd(nc, in_maps, core_ids=list(range(NCORES)))

    logits = np.empty((N, 1 + K), dtype=np.float32)
    logits[:, 0] = res.results[0]["lpos"][0]
    for i in range(NCORES):
        logits[:, 1 + i * KSH:1 + (i + 1) * KSH] = \
            res.results[i]["lneg"].reshape(N, KSH)
    labels = np.zeros(N, dtype=np.int32)
    return logits, labels


# revision 2
# speedup vs baseline: 1.5481x; 1.5481x over previous
"""MoCo grouped-queue logits kernel for Trainium2 (8 NeuronCores, Bass/Tile).

Computation (reference):
    q = l2norm(im_q @ W_q)          # [N, C]
    k = l2norm(im_k @ W_k)          # [N, C]
    l_pos[n] = q[n] . k[n]
    route[n] = (label[n] - 1) % 4
    l_neg[n, :] = q[n] @ queues[route[n]]    # [N, K]
    logits = concat([l_pos, l_neg], 1) / T   # [N, 1+K]
    labels = zeros(N)

Strategy:
  - Queues are sharded along K across the 8 cores ([4, 128, K/8] each);
    each core computes all N samples against its K-slice. Each queue
    byte is read exactly once chip-wide.
  - Samples are SORTED by route group on the host, so l_neg becomes a
    few dense [cnt<=128, 512]-tile matmuls, one group per tile — no
    masking and no 4x redundant PE work. The tile plan depends only on
    the per-group histogram; compiled programs are cached per plan.
  - The q-projection (needed by every core) is replicated; the l_pos
    path (q.k) only needs per-sample values, so its projections are
    sharded: core i computes l_pos for sorted samples [64i, 64i+64).
  - Host work is layout only: transpose/tile inputs, sort rows,
    unsort output rows.
"""

import numpy as np

# Problem constants (hardcoded; kernel.py must be self-contained).
N = 512          # batch
D = 2048         # input feature dim
C = 128          # embedding dim
K = 65536        # queue length
G = 4            # number of queues
T = 0.07         # softmax temperature
NCORES = 8
KSH = K // NCORES            # 8192 queue columns per core
DT = D // 128                # 16 contraction tiles for the projections
NLP = N // NCORES            # 64 l_pos samples per core
CW = 2048                    # queue-chunk width (columns per DMA chunk)
NKC = KSH // CW              # 4 chunks per core
NSUB = CW // 512             # 4 matmuls (N=512) per chunk

_prog_cache = {}


def _plan_from_counts(counts):
    """M-tile plan: list of (row0, cnt<=128, group) over sorted rows."""
    tiles = []
    r0 = 0
    for g in range(G):
        c, off = int(counts[g]), 0
        while off < c:
            m = min(128, c - off)
            tiles.append((r0 + off, m, g))
            off += m
        r0 += c
    return tuple(tiles)


def _build(plan):
    """Build + compile the Bass program for one tile plan."""
    import concourse.tile as tile
    from concourse import bacc, mybir

    f32 = mybir.dt.float32
    AF = mybir.ActivationFunctionType

    nc = bacc.Bacc("TRN2", target_bir_lowering=False, debug=False,
                   num_devices=NCORES)

    # Inputs, pre-tiled on host so every DMA is partition-contiguous.
    #   imqt  [128, DT*512]: [p, t*512+n] = im_q_sorted[n, t*128+p]
    #   wqt   [128, DT*128]: [p, t*128+c] = W_q[t*128+p, c]  (same wkt)
    #   imqlp/imklp [128, DT*64]: this core's 64 sorted samples
    #   qsh   [G, 128, KSH]: this core's K-slice of the queues
    imqt = nc.dram_tensor("imqt", [128, DT * 512], f32, kind="ExternalInput")
    wqt = nc.dram_tensor("wqt", [128, DT * 128], f32, kind="ExternalInput")
    wkt = nc.dram_tensor("wkt", [128, DT * 128], f32, kind="ExternalInput")
    imqlp = nc.dram_tensor("imqlp", [128, DT * NLP], f32, kind="ExternalInput")
    imklp = nc.dram_tensor("imklp", [128, DT * NLP], f32, kind="ExternalInput")
    qsh = nc.dram_tensor("qsh", [G, 128, KSH], f32, kind="ExternalInput")
    # Outputs (sorted row order): lneg [N, KSH], lpos [1, NLP] (scaled 1/T).
    lneg = nc.dram_tensor("lneg", [N, KSH], f32, kind="ExternalOutput")
    lpos = nc.dram_tensor("lpos", [1, NLP], f32, kind="ExternalOutput")

    used_groups = sorted({g for _, _, g in plan})

    with tile.TileContext(nc) as tc:
        with tc.tile_pool(name="pers", bufs=1) as pers:
            ones_col = pers.tile([128, 1], f32)
            nc.vector.memset(ones_col[:], 1.0)
            ones_row = pers.tile([1, 128], f32)
            nc.vector.memset(ones_row[:], 1.0)
            # Sorted qT scaled by invnorm/T: the stationary operand.
            qts = pers.tile([128, N], f32)

            # ---- Phase A: projections, norms, l_pos shard ----
            with tc.tile_pool(name="pa", bufs=1) as pa, \
                 tc.tile_pool(name="paps", bufs=1, space="PSUM") as paps:
                wq_sb = pa.tile([128, DT * 128], f32)
                nc.sync.dma_start(wq_sb[:], wqt[:])
                imq_sb = pa.tile([128, DT * 512], f32)
                nc.sync.dma_start(imq_sb[:], imqt[:])
                wk_sb = pa.tile([128, DT * 128], f32)
                nc.sync.dma_start(wk_sb[:], wkt[:])
                imqlp_sb = pa.tile([128, DT * NLP], f32)
                nc.sync.dma_start(imqlp_sb[:], imqlp[:])
                imklp_sb = pa.tile([128, DT * NLP], f32)
                nc.sync.dma_start(imklp_sb[:], imklp[:])

                # qT[c, n] = sum_d W_q[d, c] im_q[n, d]  (all N samples)
                qt_ps = paps.tile([128, N], f32)
                for t in range(DT):
                    nc.tensor.matmul(qt_ps[:],
                                     wq_sb[:, t * 128:(t + 1) * 128],
                                     imq_sb[:, t * 512:(t + 1) * 512],
                                     start=(t == 0), stop=(t == DT - 1))
                # l_pos shard projections (64 samples each for q and k).
                qlp_ps = paps.tile([128, NLP], f32)
                for t in range(DT):
                    nc.tensor.matmul(qlp_ps[:],
                                     wq_sb[:, t * 128:(t + 1) * 128],
                                     imqlp_sb[:, t * NLP:(t + 1) * NLP],
                                     start=(t == 0), stop=(t == DT - 1))
                klp_ps = paps.tile([128, NLP], f32)
                for t in range(DT):
                    nc.tensor.matmul(klp_ps[:],
                                     wk_sb[:, t * 128:(t + 1) * 128],
                                     imklp_sb[:, t * NLP:(t + 1) * NLP],
                                     start=(t == 0), stop=(t == DT - 1))

                qt_sb = pa.tile([128, N], f32)
                nc.vector.tensor_copy(qt_sb[:], qt_ps[:])
                qlp_sb = pa.tile([128, NLP], f32)
                nc.vector.tensor_copy(qlp_sb[:], qlp_ps[:])
                klp_sb = pa.tile([128, NLP], f32)
                nc.vector.tensor_copy(klp_sb[:], klp_ps[:])

                # Column sums over partitions via ones-vector matmuls.
                sqq = pa.tile([128, N], f32)
                nc.vector.tensor_mul(sqq[:], qt_sb[:], qt_sb[:])
                ssqq_ps = paps.tile([1, N], f32)
                nc.tensor.matmul(ssqq_ps[:], ones_col[:], sqq[:],
                                 start=True, stop=True)

                red_sb = pa.tile([128, 3 * NLP], f32)
                nc.vector.tensor_mul(red_sb[:, 0:NLP],
                                     qlp_sb[:], qlp_sb[:])
                nc.vector.tensor_mul(red_sb[:, NLP:2 * NLP],
                                     klp_sb[:], klp_sb[:])
                nc.vector.tensor_mul(red_sb[:, 2 * NLP:3 * NLP],
                                     qlp_sb[:], klp_sb[:])
                red_ps = paps.tile([1, 3 * NLP], f32)
                nc.tensor.matmul(red_ps[:], ones_col[:], red_sb[:],
                                 start=True, stop=True)

                # inv = 1 / max(sqrt(ssq), 1e-12), for q (all N).
                normq = pa.tile([1, N], f32)
                nc.scalar.activation(normq[:], ssqq_ps[:], AF.Sqrt)
                normqc = pa.tile([1, N], f32)
                nc.vector.tensor_scalar_max(normqc[:], normq[:], 1e-12)
                invq = pa.tile([1, N], f32)
                nc.vector.reciprocal(invq[:], normqc[:])
                invq_t = pa.tile([1, N], f32)
                nc.vector.tensor_scalar_mul(invq_t[:], invq[:], 1.0 / T)

                # l_pos shard: dot * invq_lp * invk_lp / T.
                norml = pa.tile([1, 2 * NLP], f32)
                nc.scalar.activation(norml[:], red_ps[:, 0:2 * NLP], AF.Sqrt)
                normlc = pa.tile([1, 2 * NLP], f32)
                nc.vector.tensor_scalar_max(normlc[:], norml[:], 1e-12)
                invl = pa.tile([1, 2 * NLP], f32)
                nc.vector.reciprocal(invl[:], normlc[:])
                lp1 = pa.tile([1, NLP], f32)
                nc.vector.tensor_mul(lp1[:], invl[:, 0:NLP], invl[:, NLP:2 * NLP])
                lp2 = pa.tile([1, NLP], f32)
                nc.vector.tensor_mul(lp2[:], lp1[:], red_ps[:, 2 * NLP:3 * NLP])
                lp3 = pa.tile([1, NLP], f32)
                nc.vector.tensor_scalar_mul(lp3[:], lp2[:], 1.0 / T)
                nc.sync.dma_start(lpos[:], lp3[:])

                # qts = qT * broadcast(invq/T): outer(ones, invq_t) via PE.
                invqb_ps = paps.tile([128, N], f32)
                nc.tensor.matmul(invqb_ps[:], ones_row[:], invq_t[:],
                                 start=True, stop=True)
                nc.vector.tensor_mul(qts[:], qt_sb[:], invqb_ps[:])

            # ---- Phase C: l_neg tiles over the queue shard ----
            with tc.tile_pool(name="qp", bufs=2) as qp, \
                 tc.tile_pool(name="sp", bufs=3) as sp, \
                 tc.tile_pool(name="cps", bufs=4, space="PSUM") as cps:
                for kc in range(NKC):
                    qch = {}
                    for g in used_groups:
                        qt_ = qp.tile([128, CW], f32, tag=f"qch{g}",
                                      name=f"qch{g}_{kc}")
                        nc.scalar.dma_start(
                            qt_[:], qsh[g, :, kc * CW:(kc + 1) * CW])
                        qch[g] = qt_
                    for ti, (r0, cnt, g) in enumerate(plan):
                        stg = sp.tile([128, CW], f32, tag="stg",
                                      name=f"stg_{kc}_{ti}")
                        for sub in range(NSUB):
                            ps = cps.tile([128, 512], f32, tag="ps",
                                          name=f"ps_{kc}_{ti}_{sub}")
                            nc.tensor.matmul(
                                ps[:cnt, :],
                                qts[:, r0:r0 + cnt],
                                qch[g][:, sub * 512:(sub + 1) * 512],
                                start=True, stop=True)
                            nc.vector.tensor_copy(
                                stg[:cnt, sub * 512:(sub + 1) * 512],
                                ps[:cnt, :])
                        nc.sync.dma_start(
                            lneg[r0:r0 + cnt, kc * CW:(kc + 1) * CW],
                            stg[:cnt, :])

    nc.compile()
    return nc


def _get_program(plan):
    if plan not in _prog_cache:
        _prog_cache[plan] = _build(plan)
    return _prog_cache[plan]


def _tile_cols(x, ncols):
    """[n, D] -> [128, DT*n] with [p, t*n+j] = x[j, t*128+p]."""
    n = x.shape[0]
    assert n == ncols
    return np.ascontiguousarray(
        x.T.reshape(DT, 128, n).transpose(1, 0, 2).reshape(128, DT * n))


def _stage_inputs(im_q, im_k, W_q, W_k, queues, label):
    f32 = np.float32
    im_q = np.asarray(im_q, dtype=f32)
    im_k = np.asarray(im_k, dtype=f32)
    W_q = np.asarray(W_q, dtype=f32)
    W_k = np.asarray(W_k, dtype=f32)
    queues = np.asarray(queues, dtype=f32)
    label = np.asarray(label)

    route = ((label.astype(np.int64) - 1) % G).astype(np.int64)
    order = np.argsort(route, kind="stable")
    counts = np.bincount(route, minlength=G)
    plan = _plan_from_counts(counts)

    im_q_s = im_q[order]
    im_k_s = im_k[order]

    imqt = _tile_cols(im_q_s, N)
    wqt = np.ascontiguousarray(
        W_q.reshape(DT, 128, C).transpose(1, 0, 2).reshape(128, DT * C))
    wkt = np.ascontiguousarray(
        W_k.reshape(DT, 128, C).transpose(1, 0, 2).reshape(128, DT * C))

    in_maps = []
    for i in range(NCORES):
        sl = slice(i * NLP, (i + 1) * NLP)
        in_maps.append({
            "imqt": imqt, "wqt": wqt, "wkt": wkt,
            "imqlp": _tile_cols(im_q_s[sl], NLP),
            "imklp": _tile_cols(im_k_s[sl], NLP),
            "qsh": np.ascontiguousarray(queues[:, :, i * KSH:(i + 1) * KSH]),
        })
    return plan, order, in_maps


def kernel(im_q, im_k, W_q, W_k, queues, label):
    from concourse.bass_utils import run_bass_kernel_spmd

    plan, order, in_maps = _stage_inputs(im_q, im_k, W_q, W_k, queues, label)
    nc = _get_program(plan)
    res = run_bass_kernel_spmd(nc, in_maps, core_ids=list(range(NCORES)))

    logits = np.empty((N, 1 + K), dtype=np.float32)
    lpos_sorted = np.concatenate(
        [res.results[i]["lpos"][0] for i in range(NCORES)])
    logits[order, 0] = lpos_sorted
    for i in range(NCORES):
        logits[order, 1 + i * KSH:1 + (i + 1) * KSH] = res.results[i]["lneg"]
    labels = np.zeros(N, dtype=np.int32)
    return logits, labels
